# revision 45
# baseline (speedup 1.0000x reference)
"""AffineLabelAttention Trainium2 kernel.

out[b, l, i, j] = W_h[l] @ head[b, i] + W_d[l] @ dep[b, j] + bias[l]

Shapes (hardcoded): head/dep [4, 1024, 768] f32, label_W [32, 1536], label_b [32].
Output [4, 32, 1024, 1024] f32 (512 MB) -> completely output-DMA-bound.

Sharding over 8 cores: core c handles batch b = c // 2 and label half
lh = c % 2 (16 labels). Each core writes a contiguous [16, 1024, 1024]
(64 MB) slice of the output.

Per-core device kernel:
  1. DMA in dep^T / head^T [768, 1024] (host pre-transposed), W halves
     transposed [768, 16], bias column [16, 1].
  2. PE matmuls: d_score[l, j] = W_d^T.T @ dep^T (bias folded in during
     PSUM evacuation), h_score[i, l] = head^T.T @ W_h^T.
  3. For each label l: replicate the d_score row across 128 partitions
     with a one-hot selector PE matmul (sel_l[16,128].T @ d_sb[16,N] ->
     [128,N] PSUM; sel_l[k,p] = (k==l), exact in fp32), evacuate to
     SBUF, then for each 128-row i-chunk a DVE/ACT tensor-scalar add of
     h_score[i_chunk, l] produces the out tile.
  4. 2 MB HWDGE DMAs stream the tiles to HBM.

  Engine-op constraint (walrus birverifier): every compute-engine operand
  (SBUF or PSUM) must start at partition 0/32/64/96 — so all per-label
  state is indexed along the free dim, never by partition offset.
"""

import sys

import numpy as np

if "/opt/trn_rl_repo" not in sys.path:
    sys.path.insert(0, "/opt/trn_rl_repo")

import concourse.bass as bass
import concourse.mybir as mybir
from concourse import bacc
from concourse.bass_utils import run_bass_kernel_spmd
from concourse.tile import TileContext

B, S, D, L = 4, 1024, 768, 32
NCORES = 8
LH = L // 2          # labels per core
KCH = D // 128       # contraction chunks (6)
ICH = S // 128       # i chunks (8)
JC = S // 512        # j chunks for d matmul (2)
IC_PER = 4           # i-chunks per output tile -> 2 MB DMAs
F32 = mybir.dt.float32
BF16 = mybir.dt.bfloat16

# knobs for test harness
TRACE = False
LAST_RESULTS = None

_CACHE = {}


def _build():
    # Bacc (not raw Bass): its compile() runs move_matmul_waits_to_ldweights
    # + generate_event_semaphores, required because TRN2 engine instructions
    # carry at most one semaphore wait.
    nc = bacc.Bacc("TRN2", target_bir_lowering=False, debug=False)
    headT = nc.dram_tensor("headT", [D, S], F32, kind="ExternalInput")
    depT = nc.dram_tensor("depT", [D, S], F32, kind="ExternalInput")
    whT = nc.dram_tensor("whT", [D, LH], F32, kind="ExternalInput")
    wdT = nc.dram_tensor("wdT", [D, LH], F32, kind="ExternalInput")
    bcol = nc.dram_tensor("bcol", [LH, 1], F32, kind="ExternalInput")
    sel = nc.dram_tensor("sel", [LH, LH * 128], F32, kind="ExternalInput")
    out = nc.dram_tensor("out", [LH, S, S], F32, kind="ExternalOutput")

    # 1 MB input chunks: chunk c covers k-slices {2c, 2c+1}
    headT_v = headT[:].rearrange("(c k p) s -> c p k s", k=2, p=128)
    depT_v = depT[:].rearrange("(c k p) s -> c p k s", k=2, p=128)
    whT_v = whT[:].rearrange("(k p) l -> p k l", p=128)       # [128, 6, 16]
    wdT_v = wdT[:].rearrange("(k p) l -> p k l", p=128)
    out_v = out[:].rearrange("l (c p) j -> l p c j", p=128)   # [16, 128, 8, 1024]

    with TileContext(nc) as tc:
        with (
            tc.tile_pool(name="const", bufs=1) as cpool,
            tc.tile_pool(name="bcast", bufs=3) as bpool,
            tc.tile_pool(name="outp", bufs=6) as opool,
            tc.tile_pool(name="psum_h", bufs=2, space="PSUM") as psh,
            tc.tile_pool(name="psum_d", bufs=1, space="PSUM") as psd,
            tc.tile_pool(name="psum_bc", bufs=4, space="PSUM") as psb,
        ):
            depT_sb = cpool.tile([128, KCH, S], F32)
            headT_sb = cpool.tile([128, KCH, S], F32)
            whT_sb = cpool.tile([128, KCH, LH], F32)
            wdT_sb = cpool.tile([128, KCH, LH], F32)
            b_col = cpool.tile([LH, 1], F32)
            sel_sb = cpool.tile([LH, LH * 128], F32)  # one-hot row selectors
            h_all = cpool.tile([128, ICH, LH], F32)   # h scores, [i, l] layout
            d_sb = cpool.tile([LH, S], F32)           # d scores + bias, [l, j]
            wu_w = cpool.tile([128, LH], BF16)        # PE warm-up operands
            wu_x = cpool.tile([128, 512], BF16)

            # Input chunks; first dep/head chunk + W first so the score
            # matmuls start as soon as chunk 0 lands.
            nc.sync.dma_start(out=depT_sb[:, 0:2, :], in_=depT_v[0])
            nc.sync.dma_start(out=wdT_sb[:], in_=wdT_v[:])
            nc.sync.dma_start(out=whT_sb[:], in_=whT_v[:])
            nc.sync.dma_start(out=headT_sb[:, 0:2, :], in_=headT_v[0])
            nc.sync.dma_start(out=b_col[:], in_=bcol[:])
            nc.sync.dma_start(out=sel_sb[:], in_=sel[:])
            for c in range(1, KCH // 2):
                nc.sync.dma_start(out=depT_sb[:, 2 * c:2 * c + 2, :],
                                  in_=depT_v[c])
                nc.sync.dma_start(out=headT_sb[:, 2 * c:2 * c + 2, :],
                                  in_=headT_v[c])

            # Scores in [l, j/i] layout: lhsT = W chunk (16-wide stationary),
            # rhs = dep/head chunk (moving, N=512). k-outer so PE consumes
            # chunk k right behind its DMA.
            d_ps = [psd.tile([LH, 512], F32, name=f"d_ps{j}") for j in range(JC)]

            # PE warm-up: HAM keeps the PE clock-gated at 1.2 GHz until it
            # sees ~3.4us of sustained matmul activity. Burn cheap bf16
            # matmuls into d_ps[0] (cleared by the first real d matmul's
            # start=True) while the input DMAs stream, so the fp32 score
            # matmuls run at 2.4 GHz.
            nc.vector.memset(wu_w[:], 0.0)
            nc.vector.memset(wu_x[:], 0.0)
            for _ in range(36):
                nc.tensor.matmul(d_ps[0][:], wu_w[:], wu_x[:],
                                 start=True, stop=True)

            # d scores first — they gate the broadcast that every output
            # tile needs. Wide form [16, 512], k-paced behind the chunks.
            for k in range(KCH):
                for jc in range(JC):
                    nc.tensor.matmul(
                        d_ps[jc][:],
                        wdT_sb[:, k, :],
                        depT_sb[:, k, jc * 512:(jc + 1) * 512],
                        start=(k == 0),
                        stop=(k == KCH - 1),
                    )
            for jc in range(JC):
                nc.vector.tensor_scalar_add(
                    d_sb[:, jc * 512:(jc + 1) * 512], d_ps[jc][:], b_col[:]
                )

            # Broadcast d row lb across 128 partitions: one-hot selector
            # matmul (exact in fp32), ACT evacuates PSUM -> SBUF.
            def bcast(lb):
                dbc = bpool.tile([128, S], F32)
                for jc in range(JC):
                    bc_ps = psb.tile([128, 512], F32)
                    nc.tensor.matmul(
                        bc_ps[:],
                        sel_sb[:, lb * 128:(lb + 1) * 128],
                        d_sb[:, jc * 512:(jc + 1) * 512],
                        start=True,
                        stop=True,
                    )
                    nc.scalar.copy(dbc[:, jc * 512:(jc + 1) * 512], bc_ps[:])
                return dbc

            dbc_next = bcast(0)

            # h scores straight into [i, l] layout: narrow matmuls
            # (lhsT = headT block [128, 128], rhs = W [128, 16]) move 8x
            # fewer PE rows than the wide form and pipeline well; first
            # 4 i-chunks gate the first output tile, so they come first.
            def h_score(ic):
                h_ps = psh.tile([128, LH], F32)
                for k in range(KCH):
                    nc.tensor.matmul(
                        h_ps[:],
                        headT_sb[:, k, ic * 128:(ic + 1) * 128],
                        whT_sb[:, k, :],
                        start=(k == 0),
                        stop=(k == KCH - 1),
                    )
                nc.scalar.copy(h_all[:, ic, :], h_ps[:])

            for ic in range(IC_PER):
                h_score(ic)
            dbc_next2 = bcast(1)
            for ic in range(IC_PER, ICH):
                h_score(ic)

            # Main loop: per-i-chunk adds of the h scalar onto the broadcast
            # d row; DVE takes ~5/7 of the adds, ACT the rest. bcast(lb+1)
            # is issued ahead of the adds so PE/ACT prefetch the next row.
            cnt = 0
            pending = [dbc_next, dbc_next2]
            for lb in range(LH):
                dbc = pending.pop(0)
                if lb + 2 < LH:
                    pending.append(bcast(lb + 2))
                for g in range(ICH // IC_PER):
                    ot = opool.tile([128, IC_PER, S], F32)
                    for s in range(IC_PER):
                        ic = g * IC_PER + s
                        scal = h_all[:, ic, lb:lb + 1]
                        if cnt % 7 < 5:
                            nc.vector.tensor_scalar_add(ot[:, s, :], dbc[:], scal)
                        else:
                            nc.scalar.add(ot[:, s, :], dbc[:], scal)
                        cnt += 1
                    nc.sync.dma_start(
                        out=out_v[lb, :, g * IC_PER:(g + 1) * IC_PER, :],
                        in_=ot[:],
                    )
    nc.compile()
    return nc


def kernel(head, dep, label_W, label_b):
    global LAST_RESULTS
    head = np.ascontiguousarray(np.asarray(head, dtype=np.float32))
    dep = np.ascontiguousarray(np.asarray(dep, dtype=np.float32))
    label_W = np.asarray(label_W, dtype=np.float32)
    label_b = np.asarray(label_b, dtype=np.float32)

    headT = np.ascontiguousarray(head.transpose(0, 2, 1))  # [B, D, S]
    depT = np.ascontiguousarray(dep.transpose(0, 2, 1))
    whT = np.ascontiguousarray(label_W[:, :D].T)           # [D, L]
    wdT = np.ascontiguousarray(label_W[:, D:].T)           # [D, L]

    # one-hot selector: sel[k, l*128 + p] = 1.0 iff k == l
    sel = np.zeros((LH, LH * 128), dtype=np.float32)
    for lb in range(LH):
        sel[lb, lb * 128:(lb + 1) * 128] = 1.0

    in_maps = []
    for c in range(NCORES):
        b, lh = divmod(c, 2)
        ls = slice(lh * LH, (lh + 1) * LH)
        in_maps.append({
            "headT": headT[b],
            "depT": depT[b],
            "whT": np.ascontiguousarray(whT[:, ls]),
            "wdT": np.ascontiguousarray(wdT[:, ls]),
            "bcol": np.ascontiguousarray(label_b[ls].reshape(LH, 1)),
            "sel": sel,
        })

    if "nc" not in _CACHE:
        _CACHE["nc"] = _build()
    nc = _CACHE["nc"]

    res = run_bass_kernel_spmd(nc, in_maps, core_ids=list(range(NCORES)),
                               trace=TRACE)
    LAST_RESULTS = res

    out = np.empty((B, L, S, S), dtype=np.float32)
    for c in range(NCORES):
        b, lh = divmod(c, 2)
        out[b, lh * LH:(lh + 1) * LH] = res.results[c]["out"]
    return out
